# revision 45
# baseline (speedup 1.0000x reference)
"""Trainium2 Bass kernel for the MHA-with-diagonal-softmax module.

Computation (per batch b):
    q = rope(x @ Wq.T), k = rope(x @ Wk.T), v = x @ Wv.T      (per head, DH=128)
    sumexp[s,h] = sum_k exp(q_h[s] . k_h[k] * DH^-0.5)
    diag[s,h]   = q_h[s] . k_h[s] * DH^-0.5
    w = exp(diag) / sumexp
    out = (w * v) @ Wo.T

Sharding: 8 cores = 2 (batch) x 4 (head groups of 4 heads).
Each core computes q/k/v for its 4 heads in transposed [head_dim, seq]
layout, the per-position softmax-diagonal weights, and its heads' part
of the output projection. Output rows 0-1023 go out as two head-pair
partials (pair-0 streams out early as stream-2/3 filler); rows
1024-2047 are single 4-head-accumulated blocks in the tail - this cuts
y traffic from 16MB to 12MB (the tail is y-DMA-bandwidth bound).

Schedule (~357us, vs 395us for the first working version):
  - phase 0 (t~9-45us): head-0 K+Q projections accumulate kb-major into
    held PSUM (K0 in 2 sco tiles, Q0 in the 3rd sco tile + both mm
    tiles) tracking the x DMA stream; x input is DMA-bound at
    ~250-310GB/s aggregate over the only 3 queues (SP/ACT HWDGE +
    gpsimd SWDGE).
  - 4 score streams (one per head, ~43-55us each): per 1024-half, 2
    score MMs + one exp with fused row-sum. ACT needs 2.66us/block
    (2x(1049ns ACTIVATE + 283ns READ_ACCUMULATOR)); PE scores are only
    0.86us/block, so every stream needs >=29us of PE filler or it goes
    ACT-bound and the PE idles on the 3-deep score-psum rotation.
  - fillers (K/Q heads 1-3, V, diag, pair-0 oproj) are paced at HALF
    grain by PE-TIME WEIGHT (chunk-half=8, diag=2, rope=0); Q3 runs as
    stream-2 filler (only stream 3 needs it) to balance streams 2/3.
Hard-won constraints this schedule encodes:
  - Tile deps are per-TILE: x must be 32 half-block tiles; head-0
    weights are 4+4 standalone tiles split/interleaved with the first x
    pieces (first MM at ~10us instead of 19us); Wv heads 0/1 REUSE the
    dead phase-0 tiles so the first V0 chunk never waits a 2MB DMA.
  - The 2-slot weight pool forces lifetimes wkt->wvB (after K3) and
    wqt->woB (after Q3); pair-0's Wo half is a standalone tile.
  - Engines run SET_ORDERING_MODE=relaxed but the ready-lookahead past
    a blocked instruction is short: latency-chained units (rope mul
    after its gpsimd swap DMA, diag after its rope) must be emitted
    behind READY PE work - rope swaps are prefetched >=1 unit ahead,
    and each builder's tail latency units are CARRIED into the next
    builder's stream (chain()).
  - A rope mul waiting at the head of the in-order DVE queue delays
    the chunk CASTs behind it, which gate the 2-buf mm psum pool,
    which starves the PE.
  - With the 2-buf mm pool, a unit holding psum across another unit's
    two allocs deadlocks the in-order PE queue: fa/fb chunk halves may
    straddle ONLY score groups (sco pool).
  - PSUM: 3x[128,1024] sco + 2x[128,512] mm = exactly 8 banks.
  - fp8: DoublePixel not implemented in this bass; DoubleRow scores
    measured slower; fp8 projections fail accuracy (5.3% vs 2% gate).
  - sbuf->sbuf SWDGE copies are ~8us/256KB of queue time - never put
    them on the critical gpsimd queue.
  - Per-core DVFS noise: chip-wide x1.2 throttle when warm, plus
    random ~150us PE-only 2.4->2.0GHz windows on individual cores;
    judge schedules by the fastest 216ns-MM-spacing core.

On-chip dtype is fp16 (same PE throughput as bf16, 8x lower rounding
error - matters because exp() amplifies absolute score error), with
fp32 PSUM accumulation everywhere.
"""

import numpy as np
from contextlib import ExitStack

# Problem constants (hardcoded per harness contract).
B, S, D, H, DH = 2, 2048, 2048, 16, 128
HPC = 4            # heads per core
NHL = HPC * DH     # 512 local head dims per core
KB = D // 128      # 16 contraction blocks
SB = S // 128      # 16 seq blocks of 128
SC = S // 512      # 4 seq/emb chunks of 512
NCORES = 8

_CACHE = {}


def _build_nc():
    import concourse.bass as bass
    import concourse.tile as tile
    from concourse import bacc, mybir
    from concourse.masks import make_identity

    F16 = mybir.dt.float16
    F32 = mybir.dt.float32
    AF = mybir.ActivationFunctionType

    # Bacc (not raw Bass): its compile() splits multi-sem waits into
    # event-semaphore instructions - HW allows at most 1 wait per inst.
    nc = bacc.Bacc("TRN2", target_bir_lowering=False, debug=False)

    # weights arrive pre-arranged partition-major on the host so each DMA
    # is 128 x 8KB contiguous descriptors (1KB-row descriptors measured
    # ~120GB/s and hog the 4-deep DMA rings)
    xT = nc.dram_tensor("xT", [D, S], F16, kind="ExternalInput").ap()
    wq = nc.dram_tensor("wq", [128, KB * 512], F16, kind="ExternalInput").ap()
    wk = nc.dram_tensor("wk", [128, KB * 512], F16, kind="ExternalInput").ap()
    wv = nc.dram_tensor("wv", [128, KB * 512], F16, kind="ExternalInput").ap()
    wo = nc.dram_tensor("wo", [128, HPC * S], F16, kind="ExternalInput").ap()
    ropeA = nc.dram_tensor("ropeA", [128, S], F16, kind="ExternalInput").ap()
    ropeB = nc.dram_tensor("ropeB", [128, S], F16, kind="ExternalInput").ap()
    y = nc.dram_tensor("y", [2, S, D], F16, kind="ExternalOutput").ap()

    xT_r = xT.rearrange("(a p) s -> a p s", p=128)
    # weights are head-major [p, mt, kb, m] so the prologue can pull just
    # head 0's contiguous slab first
    wq_p = wq.rearrange("p (h a m) -> p h a m", h=HPC, a=KB)
    wk_p = wk.rearrange("p (h a m) -> p h a m", h=HPC, a=KB)
    wv_p = wv.rearrange("p (h a m) -> p h a m", h=HPC, a=KB)
    wo_p = wo.rearrange("p (h n) -> p h n", h=HPC)

    with tile.TileContext(nc) as tc, ExitStack() as ctx:
        pool = ctx.enter_context(tc.tile_pool(name="sb", bufs=1))
        pp = ctx.enter_context(tc.tile_pool(name="ps", bufs=1, space="PSUM"))

        # ---- constants (gpsimd SWDGE: small, keeps HWDGE queues for x/w) --
        # pre-expanded [128, S] rope operands: ra = [cos; cos],
        # rb = [-sin; sin]. (Building them on-chip from [64, S] halves was
        # tried and is a net LOSS: a 256KB sbuf->sbuf SWDGE copy takes ~8us
        # of gpsimd queue time and delays the x pieces + rope swaps behind
        # it far more than the 0.5MB of saved HBM reads.)
        ra = pool.tile([128, S], F16, name="ra")
        rb = pool.tile([128, S], F16, name="rb")
        nc.gpsimd.dma_start(ra[:, :], ropeA[:, :])
        nc.gpsimd.dma_start(rb[:, :], ropeB[:, :])
        ident = pool.tile([128, 128], F32, name="ident")
        make_identity(nc, ident[:, :])
        ones1 = pool.tile([128, 128], F16, name="ones1")
        nc.gpsimd.memset(ones1[:, :], 1.0)

        # ---- big input DMAs ----
        # Head-0 K/Q weights go FIRST and in their OWN tiles: deps are
        # per-tile, so folding them into the 4-head weight tiles would gate
        # phase 0's first matmul on the heads-1-3 slabs queued behind all
        # of x (measured: first matmul at 19us instead of ~9us).
        # x is one half-block tile per (kb, col-half) so the head-0
        # projections can accumulate into held PSUM tiles as each piece
        # lands. Everything not needed before the first score block
        # (heads 1-3 weights) queues AFTER x: input DMA runs at ~310GB/s
        # aggregate, so every early byte delays phase 0's critical path.
        # head-0 weights split into 4 tiles of 4 kb-slabs each,
        # interleaved with the first x pieces: the first matmul needs only
        # wk0s[0]+xh0 (384KB on the sync queue) instead of the whole 768KB
        wk0_r = wk_p[:, 0].rearrange("p (j b) m -> p j b m", j=4)
        wq0_r = wq_p[:, 0].rearrange("p (j b) m -> p j b m", j=4)
        wk0s = [pool.tile([128, 4, 128], F16, name=f"wk0{j}")
                for j in range(4)]
        wq0s = [pool.tile([128, 4, 128], F16, name=f"wq0{j}")
                for j in range(4)]
        nc.sync.dma_start(wk0s[0][:, :, :], wk0_r[:, 0])
        nc.scalar.dma_start(wq0s[0][:, :, :], wq0_r[:, 0])

        xh = [pool.tile([128, 1024], F16, name=f"xh{i}") for i in range(32)]

        def xsb(kb, sc):
            # x chunk (kb, sc) as a [128, 512] slice of its half-tile
            return xh[2 * kb + sc // 2][:, (sc % 2) * 512:(sc % 2 + 1) * 512]

        # piece->queue assignment: phase 0 consumes pieces in kb order, so
        # the first 3 kb go to the HWDGE queues only (the gpsimd queue
        # starts ~3us later and carries ra/rb first; a gpsimd piece at
        # kb 1-2 head-of-line stalls the phase-0 loop ~5us). gpsimd's 8
        # pieces spread over kb 3-15. The remaining wk0/wq0 slabs
        # interleave into the first sync/scalar x pieces.
        engs = [nc.sync, nc.scalar, nc.gpsimd]
        easg = [0, 1] * 3 + [2, 0, 1] * 8 + [0, 1]
        assert len(easg) == 32 and easg.count(2) == 8
        # remaining weight slabs are needed at kb 4/8/12 (PE reaches them
        # ~3.5/7/10.5us after the first MM) - interleave them after x
        # pieces 2/4/6 of each HWDGE queue so x pieces for kb 1-2 land
        # first
        nsync = nscal = 0
        for i in range(32):
            e = easg[i]
            kb, half = i // 2, i % 2
            engs[e].dma_start(
                xh[i][:, :], xT_r[kb][:, half * 1024:(half + 1) * 1024])
            if e == 0:
                nsync += 1
                if nsync in (3, 5, 7):
                    nc.sync.dma_start(wk0s[(nsync - 1) // 2][:, :, :],
                                      wk0_r[:, (nsync - 1) // 2])
            elif e == 1:
                nscal += 1
                if nscal in (3, 5, 7):
                    nc.scalar.dma_start(wq0s[(nscal - 1) // 2][:, :, :],
                                        wq0_r[:, (nscal - 1) // 2])
        # heads 1-3 K/Q weights ([:, 0:3] of the 2MB-class "w" pool slabs;
        # sized like wvt/wot so the tag pool slots are uniform)
        wkt = pool.tile([128, HPC, KB, 128], F16, name="wt", tag="w", bufs=2)
        wqt = pool.tile([128, HPC, KB, 128], F16, name="wt", tag="w", bufs=2)
        nc.sync.dma_start(wkt[:, 0:3], wk_p[:, 1:4])
        nc.scalar.dma_start(wqt[:, 0:3], wq_p[:, 1:4])

        # ---- persistent q/k/v head tiles ([head_dim, seq] layout) ----
        qh = [pool.tile([128, S], F16, name=f"qh{h}") for h in range(HPC)]
        kh = [pool.tile([128, S], F16, name=f"kh{h}") for h in range(HPC)]
        vh = [pool.tile([128, S], F16, name=f"vh{h}") for h in range(HPC)]

        # per-head row vectors live at partition 32*h (engine ops only
        # support start partitions that are multiples of 32)
        ds_diag = pool.tile([128, S], F16, name="ds_diag")
        ds_sum = pool.tile([128, S], F16, name="ds_sum")
        w4 = pool.tile([128, S], F16, name="w4")
        # 2 accum columns per sq block (exp is done in 1024-wide halves)
        sumf = [pool.tile([128, 2 * SB], F32, name=f"sumf{h}")
                for h in range(HPC)]

        def proj_mms(wt, wi, sc):
            # (wt[:, wi] block).T @ x chunk -> a rotating psum tile
            ps = pp.tile([128, 512], F32, name="mmps", tag="mm", bufs=2)
            for kb in range(KB):
                nc.tensor.matmul(
                    ps[:, :], wt[:, wi, kb, :], xsb(kb, sc),
                    start=(kb == 0), stop=(kb == KB - 1))
            return ps

        def rope_pre(dst, c):
            # half-swap prefetch: issue the gpsimd sbuf->sbuf swap early
            # so the DVE rope muls never head-of-line block the in-order
            # DVE queue (a blocked rope mul delays the chunk CASTs behind
            # it, which gate the mm psum pool, which starves the PE).
            sl = slice(c * 1024, (c + 1) * 1024)
            # SWDGE (gpsimd) keeps this 1 queue -> 1 sem; a wide HWDGE
            # sbuf->sbuf DMA fans out over many queues and blows the
            # consumer's sync-wait slot budget.
            swp = pool.tile([128, 1024], F16, name="swp", tag="swp", bufs=2)
            nc.gpsimd.dma_start(swp[0:64, :], dst[64:128, sl])
            nc.gpsimd.dma_start(swp[64:128, :], dst[0:64, sl])
            return swp

        def rope_mul(dst, c, swp):
            # dst half (in place): top = te*cos - to*sin ; bot = te*sin+to*cos
            # ra = [cosT; cosT], rb = [-sinT; sinT]; swap = halves exchanged.
            sl = slice(c * 1024, (c + 1) * 1024)
            u = pool.tile([128, 1024], F16, name="u", tag="sc", bufs=2)
            nc.vector.tensor_mul(u[:, :], dst[:, sl], ra[:, sl])
            v2 = pool.tile([128, 1024], F16, name="v2", tag="sc", bufs=2)
            nc.vector.tensor_mul(v2[:, :], swp[:, :], rb[:, sl])
            nc.vector.tensor_add(dst[:, sl], u[:, :], v2[:, :])

        def rope_half(dst, c):
            rope_mul(dst, c, rope_pre(dst, c))

        def diag_half(h, c):
            # ds_diag[32h, s-half] = sum_m qh[h][m,s] * kh[h][m,s]
            hp = 32 * h
            sl = slice(c * 1024, (c + 1) * 1024)
            pr = pool.tile([128, 1024], F16, name="pr", tag="pr", bufs=2)
            nc.vector.tensor_mul(pr[:, :], qh[h][:, sl], kh[h][:, sl])
            for cc in range(2):
                dps = pp.tile([128, 512], F32, name="mmps", tag="mm", bufs=2)
                nc.tensor.matmul(dps[:, :], ones1[:, :],
                                 pr[:, cc * 512:(cc + 1) * 512],
                                 start=True, stop=True)
                o = (2 * c + cc) * 512
                nc.vector.tensor_copy(ds_diag[hp:hp + 1, o:o + 512],
                                      dps[hp:hp + 1, :])

        # ====== scores stream ======
        ex = pool.tile([128, 1024], F16, name="ex")

        def scores_half(h, sq, half):
            # one 1024-half: 2 score MMs into a rotating [128,1024] psum
            # (3-buffered so the PE can run ~1.5 blocks ahead of the exp
            # stream), one exp with fused row-sum. (Row-sum via DVE
            # tensor_reduce was tried: 1.2us/half on DVE - far worse than
            # ACT's 283ns READ_ACCUMULATOR.)
            sps = pp.tile([128, 1024], F32, name="sps", tag="sco",
                          bufs=3)
            for cc in range(2):
                ck = 2 * half + cc
                nc.tensor.matmul(sps[:, cc * 512:(cc + 1) * 512],
                                 qh[h][:, sq * 128:(sq + 1) * 128],
                                 kh[h][:, ck * 512:(ck + 1) * 512],
                                 start=True, stop=True)
            col = half * SB + sq
            nc.scalar.activation(ex[:, :], sps[:, :], AF.Exp,
                                 accum_out=sumf[h][:, col:col + 1])

        rsq = {}

        def head_sum_pre(h):
            # DVE-only piece: sum the 2 half-accums, reciprocal
            stot = pool.tile([128, SB], F32, name="stot", tag="rs", bufs=2)
            nc.vector.tensor_add(stot[:, :], sumf[h][:, 0:SB],
                                 sumf[h][:, SB:2 * SB])
            rsq[h] = pool.tile([128, SB], F32, name="rs", tag="rs", bufs=2)
            nc.vector.reciprocal(rsq[h][:, :], stot[:, :])

        def head_sum_post(h):
            # transpose -> [1,S] ds_sum row (PE piece, emitted after a
            # filler so the PE queue has work while the DVE piece resolves)
            hp = 32 * h
            tps = pp.tile([16, 128], F32, name="mmps", tag="mm", bufs=2)
            nc.tensor.transpose(tps[:, :], rsq[h][:, :], ident[:, :])
            st = pool.tile([16, 128], F16, name="st", tag="st", bufs=2)
            nc.vector.tensor_copy(st[:, :], tps[:, :])
            nc.gpsimd.dma_start(ds_sum[hp:hp + 1, :], st[:, :])

        def pair_head(p, units):
            # w = exp(diag) * recip(sumexp); attn = w (bcast) * v, into kh.
            # ACT exps first (no PE coupling), then broadcast+scale chunk
            # groups interleaved with independent PE units.
            for h in (2 * p, 2 * p + 1):
                hp = 32 * h
                nc.scalar.activation(w4[hp:hp + 1, :],
                                     ds_diag[hp:hp + 1, :], AF.Exp)
            for _ in range(2):
                if units:
                    units.pop(0)()
            for h in (2 * p, 2 * p + 1):
                hp = 32 * h
                nc.vector.tensor_mul(w4[hp:hp + 1, :], w4[hp:hp + 1, :],
                                     ds_sum[hp:hp + 1, :])
            for ck in range(SC):
                for h in (2 * p, 2 * p + 1):
                    hp = 32 * h
                    # K=1 outer-product broadcast of the w row to 128 parts
                    bps = pp.tile([128, 512], F32, name="mmps", tag="mm",
                                  bufs=2)
                    nc.tensor.matmul(bps[:, :], ones1[hp:hp + 1, :],
                                     w4[hp:hp + 1, ck * 512:(ck + 1) * 512],
                                     start=True, stop=True,
                                     tile_position=(hp, 0))
                    # attn scaling straight from psum (no bounce buffer)
                    nc.vector.tensor_mul(kh[h][:, ck * 512:(ck + 1) * 512],
                                         bps[:, :],
                                         vh[h][:, ck * 512:(ck + 1) * 512])
                for _ in range(2):
                    if units:
                        units.pop(0)()

        # output-projection modes: the tail is y-DMA-BANDWIDTH bound, so
        # only blocks sb 0-7 use the 2-partial scheme (pair-0 partial
        # streams out early as stream-2/3 filler, pair-1 partial in the
        # tail); blocks sb 8-15 are computed in the tail as single
        # 4-head-accumulated FULL blocks: y traffic drops from 16MB to
        # 12MB and tail traffic from ~10MB to ~8MB.
        OPROJ_HEADS = {0: (0, 1), 1: (2, 3), 2: (0, 1, 2, 3)}

        def oproj_unit(p, sb, ncx, yts, copy_eng, ptag="mm"):
            # one 128-row x 512-col chunk of the output projection
            heads = OPROJ_HEADS[p]
            ps = pp.tile([128, 512], F32, name="mmps", tag=ptag,
                         bufs=2 if ptag == "mm" else 3)
            for i, h in enumerate(heads):
                wt = woA if h < 2 else woB
                nc.tensor.matmul(
                    ps[:, :], kh[h][:, sb * 128:(sb + 1) * 128],
                    wt[:, h % 2, ncx * 512:(ncx + 1) * 512],
                    start=(i == 0), stop=(i == len(heads) - 1))
            dst = yts[:, ncx * 512:(ncx + 1) * 512]
            if copy_eng == "act":
                nc.scalar.activation(dst, ps[:, :], AF.Copy)
            else:
                nc.vector.tensor_copy(dst, ps[:, :])
            if ncx % 2 == 1:
                # y DMA in two 256KB halves per block (after ncx 1 and 3)
                # so the 2-buf yts tile releases ~1.4us earlier. Blocks
                # alternate the two HWDGE queues. During the streams ACT
                # is running the exps, so pair-0 blocks stay on SP.
                eng = nc.sync if (p == 0 or sb % 2 == 0) else nc.scalar
                row = 1 if p == 1 else 0
                h0 = (ncx - 1) * 512
                eng.dma_start(
                    y[row, sb * 128:(sb + 1) * 128, h0:h0 + 1024],
                    yts[:, h0:h0 + 1024])

        # ================= emission =================
        # Phase 0: head-0 K and Q projections in (kb, col-half)-major
        # order - one accumulation step into held PSUM tiles (K0 in two
        # sco tiles, Q0 in the 3rd sco tile + the two mm-pool tiles) per
        # x piece as it lands, so the PE tracks the x DMA stream instead
        # of waiting for all of x.
        kA = pp.tile([128, 1024], F32, name="sps", tag="sco", bufs=3)
        kB = pp.tile([128, 1024], F32, name="sps", tag="sco", bufs=3)
        qA = pp.tile([128, 1024], F32, name="sps", tag="sco", bufs=3)
        qm = [pp.tile([128, 512], F32, name="mmps", tag="mm", bufs=2)
              for _ in range(2)]
        for kb in range(KB):
            st_, sp_ = (kb == 0), (kb == KB - 1)
            for sc in range(4):
                kt = kA if sc < 2 else kB
                nc.tensor.matmul(kt[:, (sc % 2) * 512:(sc % 2 + 1) * 512],
                                 wk0s[kb // 4][:, kb % 4, :], xsb(kb, sc),
                                 start=st_, stop=sp_)
                qt = (qA[:, (sc % 2) * 512:(sc % 2 + 1) * 512]
                      if sc < 2 else qm[sc - 2][:, :])
                nc.tensor.matmul(qt, wq0s[kb // 4][:, kb % 4, :],
                                 xsb(kb, sc), start=st_, stop=sp_)
        # drain + rope, interleaved with head-1 K projection chunks so the
        # PE has queued work while the DVE/gpsimd rope chain resolves. Q0's
        # mm-pool psums are drained before the K1 chunks rotate onto their
        # banks; the K1 copies are emitted AFTER the rope ops (DVE is
        # in-order, so the reverse would head-of-line block the ropes).
        nc.vector.tensor_copy(qh[0][:, 1024:1536], qm[0][:, :])
        nc.vector.tensor_copy(qh[0][:, 1536:2048], qm[1][:, :])
        nc.vector.tensor_copy(kh[0][:, 0:1024], kA[:, :])
        p0 = proj_mms(wkt, 0, 0)
        rope_half(kh[0], 0)
        nc.vector.tensor_copy(kh[0][:, 1024:2048], kB[:, :])
        nc.vector.tensor_copy(kh[1][:, 0:512], p0[:, :])
        p1 = proj_mms(wkt, 0, 1)
        rope_half(kh[0], 1)
        nc.vector.tensor_copy(qh[0][:, 0:1024], qA[:, :])
        nc.vector.tensor_copy(kh[1][:, 512:1024], p1[:, :])
        p2 = proj_mms(wkt, 0, 2)
        rope_half(qh[0], 0)
        nc.vector.tensor_copy(kh[1][:, 1024:1536], p2[:, :])
        p3 = proj_mms(wkt, 0, 3)
        rope_half(qh[0], 1)
        nc.vector.tensor_copy(kh[1][:, 1536:2048], p3[:, :])

        # Filler micro-units (~1.7us of PE each), emitted between score
        # matmul groups. Small units distribute evenly into the ~2us of PE
        # slack per score block; a monolithic 3.5us chunk can't. Each proj
        # chunk is two halves sharing one psum tile (held across the gap);
        # ropes/diags are standalone units. Order respects cross-engine
        # in-order queues (an instruction emitted before its producer
        # would head-of-line block its engine).
        def chunk_units(wsl, dests, mt, sc):
            # wsl(kb) -> [128,128] stationary weight slice. Two ~1.7us
            # halves sharing one mm psum tile. With the 2-buf mm pool,
            # another mm-pool alloc between fa and fb would deadlock the
            # in-order PE queue, so the scheduler only lets score groups
            # (sco pool) run between them.
            cell = []

            def fa():
                ps = pp.tile([128, 512], F32, name="mmps", tag="mm", bufs=2)
                cell.append(ps)
                for kb in range(KB // 2):
                    nc.tensor.matmul(
                        ps[:, :], wsl(kb), xsb(kb, sc),
                        start=(kb == 0), stop=False)

            def fb():
                ps = cell[0]
                for kb in range(KB // 2, KB):
                    nc.tensor.matmul(
                        ps[:, :], wsl(kb), xsb(kb, sc),
                        start=False, stop=(kb == KB - 1))
                nc.vector.tensor_copy(
                    dests[mt][:, sc * 512:(sc + 1) * 512], ps[:, :])
            return [("fa", 8, fa), ("fb", 8, fb)]

        def rope_units(dst, c):
            # (pre, mul) unit pair; schedule pre >=1 unit before mul so
            # the swap DMA latency is covered by PE work
            cell = []

            def pre():
                cell.append(rope_pre(dst, c))

            def mul():
                rope_mul(dst, c, cell[0])
            return [("atom", 0, pre)], [("atom", 0, mul)]

        def diag_units(h):
            return [("atom", 2, lambda: diag_half(h, 0)),
                    ("atom", 2, lambda: diag_half(h, 1))]

        wvB = None
        woB = None
        # pair-0's half of Wo lives in a standalone 1MB tile so it can
        # load during stream 1 (the 2-slot "w" pool's wqt slot is only
        # free after Q3, which now runs as stream-2 filler); pair-1's
        # half takes the wqt slot right after Q3.
        woA = pool.tile([128, 2, S], F16, name="woA")

        # Wv heads 0/1 REUSE the dead phase-0 weight tiles (wk0s/wq0s):
        # no pool-rotation constraint, so they load early in stream 0 and
        # the first V0 chunk never waits on a 2MB transfer. Heads 2/3
        # take the wkt pool slot right after K3 is consumed.
        wv_r = wv_p.rearrange("p h (j b) m -> p h j b m", j=4)

        def load_wv0():
            for j in range(4):
                nc.sync.dma_start(wk0s[j][:, :, :], wv_r[:, 0, j])

        def load_wv1():
            for j in range(4):
                nc.scalar.dma_start(wq0s[j][:, :, :], wv_r[:, 1, j])

        def load_wvB():
            nonlocal wvB
            wvB = pool.tile([128, 2, KB, 128], F16, name="wt", tag="w",
                            bufs=2)
            nc.sync.dma_start(wvB[:, :, :, :], wv_p[:, 2:4])

        def load_woA():
            nc.sync.dma_start(woA[:, :, :], wo_p[:, 0:2])

        def load_woB():
            nonlocal woB
            woB = pool.tile([128, 2, S], F16, name="wt", tag="w", bufs=2)
            nc.sync.dma_start(woB[:, :, :], wo_p[:, 2:4])

        VW = [lambda kb: wk0s[kb // 4][:, kb % 4, :],
              lambda kb: wq0s[kb // 4][:, kb % 4, :],
              lambda kb: wvB[:, 0, kb, :],
              lambda kb: wvB[:, 1, kb, :]]

        def v_units(mt):
            units = []
            for sc in range(SC):
                units += chunk_units(VW[mt], vh, mt, sc)
            return units

        def chain(groups):
            # flatten (units, carry) builder outputs; each group's carry
            # (latency-chained zero/low-PE units) lands after the FIRST
            # chunk pair of the NEXT group so their deps resolve behind
            # ready PE work
            out, carry = [], []
            for g in groups:
                units, newcarry = g if isinstance(g, tuple) else (g, [])
                items, i = [], 0
                while i < len(units):
                    if units[i][0] == "fa":
                        items.append([units[i], units[i + 1]])
                        i += 2
                    else:
                        items.append([units[i]])
                        i += 1
                if items:
                    out += items[0]
                    out += carry
                    for it in items[1:]:
                        out += it
                    carry = newcarry
                else:
                    out += carry
                    carry = newcarry
            return out + carry

        def k_units(mt):
            # tail latency units (rope mul of the last half) are CARRIED
            # into the next builder's stream: emitted right after a ready
            # chunk pair so the PE never faces a run of dep-chained
            # zero-PE units (the relaxed-order engines have only a short
            # ready-lookahead past a blocked instruction)
            pre0, mul0 = rope_units(kh[mt], 0)
            pre1, mul1 = rope_units(kh[mt], 1)
            units = []
            for sc in range(SC):
                units += chunk_units(
                    lambda kb, mt=mt: wkt[:, mt - 1, kb, :], kh, mt, sc)
                if sc == 1:
                    units += pre0
                if sc == 2:
                    units += mul0
                if sc == 3:
                    units += pre1
            return units, mul1

        def q_units(mt):
            pre0, mul0 = rope_units(qh[mt], 0)
            pre1, mul1 = rope_units(qh[mt], 1)
            units = []
            for sc in range(SC):
                units += chunk_units(
                    lambda kb, mt=mt: wqt[:, mt - 1, kb, :], qh, mt, sc)
                if sc == 1:
                    units += pre0
                if sc == 2:
                    units += mul0
                if sc == 3:
                    # diag half-0 here: its pr-mul chain through mul0
                    # resolved during the sc3 chunk
                    units.append(("atom", 2, lambda mt=mt: diag_half(mt, 0)))
                    units += pre1
            return units, (mul1
                           + [("atom", 2, lambda mt=mt: diag_half(mt, 1))])

        # Flat micro-unit lists with PE-time-weighted proportional
        # pacing. Weights are PE cost in 512-col-MM units: pacing by PE
        # TIME (not unit count) keeps every score slot fed with >=1.8us
        # of PE filler - slots paced onto rope/diag units (little or no
        # PE content) starve the PE and it stalls on the score psum
        # rotation within ~3 halves.
        # Stream budget: ACT needs 2.66us/sq (two 1024-wide exps + two
        # accumulator reads); PE scores are only 0.86us/sq, so each of
        # the 4 streams needs >=29us (~132 weight) of filler or it goes
        # ACT-bound and the PE idles. Q3 therefore moves from stream 2's
        # prerequisite set into its filler list (it is only needed by
        # stream 3), balancing streams 2/3 that would otherwise be ~15us
        # ACT-bound while streams 0/1 sit on surplus PE work.
        def atom(f):
            return ("atom", 0, f)

        k1p0, k1m0 = rope_units(kh[1], 0)
        k1p1, k1m1 = rope_units(kh[1], 1)
        fill_a = (k1p0 + k1p1
                  + [("atom", 2, lambda: diag_half(0, 0)),
                     ("atom", 2, lambda: diag_half(0, 1))]
                  + k1m0 + k1m1
                  + chain([([atom(load_wv0)], []), q_units(1),
                           ([atom(load_wv1)], []), k_units(2),
                           q_units(2), k_units(3),
                           ([atom(load_wvB), atom(load_woA)], []),
                           v_units(0), v_units(1)]))

        def pop_units(fill, n):
            # run n units, treating an fa/fb pair as back-to-back (safe:
            # equivalent to a whole chunk)
            done = 0
            while done < n and fill:
                kind, w, f = fill.pop(0)
                f()
                done += 1
                if kind == "fa":
                    fill.pop(0)[2]()
                    done += 1

        yts = {}
        ofill = [(0, sb, ncx) for sb in range(SB // 2) for ncx in range(SC)]

        def oproj_pop(n, copy_eng="dve", keep=0):
            for _ in range(n):
                if len(ofill) <= keep:
                    return
                p, sb, ncx = ofill.pop(0)
                if ncx == 0:
                    yts[p] = pool.tile([128, S], F16, name="yt",
                                       tag="yt", bufs=2)
                oproj_unit(p, sb, ncx, yts[p], copy_eng)

        def stream(h, fill, frac=1.0, per_sq_oproj=0):
            # scores BEFORE the slot's fillers: Bacc lowers cross-engine
            # deps as monotonic queue-count gates, so an exp emitted after
            # a filler would wait for that filler's DVE copy too.
            # Fillers are paced at HALF grain (32 points per stream) by
            # PE weight, so every ~1.33us of ACT exp work gets matched PE
            # filler - slot-level pacing alternated heavy/light slots and
            # the light ones stalled the PE on the score psum rotation.
            # An open fa is closed (fb) right after the next score group,
            # so chunk halves interleave the score stream at ~1.7us grain
            # without any other mm alloc between them.
            take = int(round(sum(u[1] for u in fill) * frac))
            taken = 0
            opn = None
            for sq in range(SB):
                for half in range(2):
                    scores_half(h, sq, half)
                    if opn is not None:
                        opn()
                        opn = None
                        taken += 8
                    if half == 0 and per_sq_oproj:
                        # hold back 4 pair-0 units to feed the PE through
                        # the hsp(3)/pair-1 boundary chain
                        oproj_pop(per_sq_oproj, keep=4)
                    tgt = take * (2 * sq + half + 1) // (2 * SB)
                    while taken < tgt and fill and opn is None:
                        kind, w, f = fill.pop(0)
                        f()
                        taken += w
                        if kind == "fa":
                            opn = fill.pop(0)[2]
            if opn is not None:
                opn()
            head_sum_pre(h)

        # the last V1 units migrate into the pair-0 boundary (PE work
        # that covers the hst/pair serial chain). Taking only V1 units
        # (not V2): stealing from stream-2's filler list left it
        # ACT-bound by ~7us.
        vb = fill_a[-6:]
        del fill_a[-6:]

        stream(0, fill_a, frac=0.5)
        pop_units(fill_a, 3)
        head_sum_post(0)
        stream(1, fill_a)
        # stream-2 filler: Q3 (only needed by stream 3) + half of V2 + the
        # pair-1 wo load; V2's other half moves to stream 3 (which lost
        # its oproj filler stock to the tail restructure)
        v2u = v_units(2)
        v2_late = v2u[4:]
        fill_b = chain([q_units(3), ([atom(load_woB)], []), v2u[:4]])
        # ~3.4us of PE cover BEFORE the transpose: head_sum_post's PE
        # piece waits on the recip which waits on the head's LAST exps
        # (still ~5us behind the last score matmuls), and the PE queue
        # would stall behind it
        pop_units(vb, 2)
        head_sum_post(1)
        vb_f = []
        while vb:
            kind, w, f = vb.pop(0)
            if kind == "fa":
                f2 = vb.pop(0)[2]
                vb_f.append(lambda f=f, f2=f2: (f(), f2()))
            else:
                vb_f.append(f)
        pair_head(0, vb_f)
        for f in vb_f:
            f()
        hold2 = fill_b[-2:]
        del fill_b[-2:]
        stream(2, fill_b, per_sq_oproj=1)
        # cover the hsp(2) transpose chain (recip waits stream-2's last
        # exps, ~2.7us behind the last score MMs)
        pop_units(hold2, 2)
        oproj_pop(2, keep=2)
        head_sum_post(2)
        # V3 + half of V2 (needed only by pair_head(1)) keep the PE fed
        # while the score psum rotation serializes against each block's
        # exp; 2 held-back V3 chunks + the last pair-0 oproj units cover
        # the hsp(3)/pair-1 chain
        s3fill = v2_late + v_units(3)
        hold3 = s3fill[-4:]
        del s3fill[-4:]
        stream(3, s3fill, per_sq_oproj=1)

        # tail list: pair-1 partials for sb 0-7, full 4-head blocks for
        # sb 8-15 (their attn chunks become valid ck-group by ck-group
        # inside pair_head(1), and units pop in sb order, so reserve pops
        # after ck group g only touch sb <= 4g+3)
        otail = ([(1, sb, ncx) for sb in range(SB // 2)
                  for ncx in range(SC)]
                 + [(2, sb, ncx) for sb in range(SB // 2, SB)
                    for ncx in range(SC)])
        ti = [0]

        def tail_pop(n):
            for _ in range(n):
                if not otail:
                    return
                p, sb, ncx = otail.pop(0)
                if ncx == 0:
                    yts[p] = pool.tile([128, S], F16, name="yt", tag="yt",
                                       bufs=2)
                i = ti[0]
                oproj_unit(p, sb, ncx, yts[p], "act" if i % 2 else "dve",
                           ptag="mm" if i % 2 else "sco")
                ti[0] += 1

        def reserve_unit():
            def f():
                tail_pop(1)
            return f

        def p0_unit():
            def f():
                oproj_pop(1)
            return f

        pop_units(hold3, 2)
        head_sum_post(3)
        pop_units(hold3, 2)
        # pair_head's pre-mul and first-ck cover pops run BEFORE most
        # attn chunks are written, so they must be pair-0 units (kh0/kh1
        # attn is valid); only later per-ck pops may pull tail units
        # (their sb index grows slower than the completed ck range)
        pair_head(1, [p0_unit() for _ in range(4)]
                  + [reserve_unit() for _ in range(6)])

        # tail: psum->sbuf copies alternate DVE/ACT (ACT is idle by now),
        # and units alternate between the mm pool and the now-idle sco
        # banks so copies never gate matmuls.
        while otail:
            tail_pop(1)

    nc.compile()
    return nc


def _get_nc():
    if "nc" not in _CACHE:
        _CACHE["nc"] = _build_nc()
    return _CACHE["nc"]


_PERM = np.concatenate([np.arange(0, DH, 2), np.arange(1, DH, 2)])


def _host_inputs(x, rope_cos, rope_sin, Wq, Wk, Wv, Wo):
    """Build the 8 per-core input maps."""
    f16 = np.float16
    cosT = np.ascontiguousarray(np.asarray(rope_cos, np.float32)[0, :, 0, :].T)
    sinT = np.ascontiguousarray(np.asarray(rope_sin, np.float32)[0, :, 0, :].T)
    ra = np.concatenate([cosT, cosT], 0).astype(f16)
    rb = np.concatenate([-sinT, sinT], 0).astype(f16)

    Wq = np.asarray(Wq, np.float32)
    Wk = np.asarray(Wk, np.float32)
    Wv = np.asarray(Wv, np.float32)
    Wo = np.asarray(Wo, np.float32)
    x = np.asarray(x, np.float32)

    xTb = [np.ascontiguousarray(x[b].T).astype(f16) for b in range(B)]
    scale = DH ** -0.5

    def pm(arr, nblk):
        # partition-major DMA layout: [p, blk*inner + m] = arr[blk*128+p, m]
        inner = arr.shape[1]
        return np.ascontiguousarray(
            arr.reshape(nblk, 128, inner).transpose(1, 0, 2)
            .reshape(128, nblk * inner))

    def pm_head(arr):
        # head-major layout: [p, ((mt*KB + a)*128 + m)] = arr[a*128+p, mt*128+m]
        return np.ascontiguousarray(
            arr.reshape(KB, 128, HPC, 128).transpose(1, 2, 0, 3)
            .reshape(128, HPC * KB * 128))

    in_maps = []
    for core in range(NCORES):
        b, g = divmod(core, HPC)
        hs = g * HPC
        rows = np.concatenate(
            [h * DH + _PERM for h in range(hs, hs + HPC)])      # deinterleave
        rows_v = np.arange(hs * DH, (hs + HPC) * DH)
        in_maps.append({
            "xT": xTb[b],
            "wq": pm_head((Wq[rows] * scale).T).astype(f16),
            "wk": pm_head(Wk[rows].T).astype(f16),
            "wv": pm_head(Wv[rows_v].T).astype(f16),
            "wo": pm(Wo[:, rows_v].T, HPC).astype(f16),
            "ropeA": ra,
            "ropeB": rb,
        })
    return in_maps


def kernel(x, rope_cos, rope_sin, Wq, Wk, Wv, Wo, _trace=False, _trace_cores=None):
    from concourse.bass_utils import run_bass_kernel_spmd

    nc = _get_nc()
    in_maps = _host_inputs(x, rope_cos, rope_sin, Wq, Wk, Wv, Wo)
    res = run_bass_kernel_spmd(nc, in_maps, list(range(NCORES)),
                               trace=_trace, trace_cores=_trace_cores)
    _CACHE["last_result"] = res

    out = np.zeros((B, S, D), np.float32)
    for core in range(NCORES):
        b = core // HPC
        yv = res.results[core]["y"].astype(np.float32)
        # y[0]: pair-0 partial for rows 0-1023, FULL blocks for rows
        # 1024-2047; y[1]: pair-1 partial for rows 0-1023 only
        out[b] += yv[0]
        out[b, :S // 2] += yv[1][:S // 2]
    return out

